# revision 34
# baseline (speedup 1.0000x reference)
"""Trainium2 Bass kernel for BaseBertSelfAttention (B=2, S=2048, H=1024, 16 heads).

Sharding (8 NeuronCores):
  - Tensor-parallel on heads: core c owns heads (2c, 2c+1) -> d_local = 128.
  - Each core: QKV projections (column-parallel) for its 2 heads over BOTH
    batches, attention in transposed layout (scores^T: keys on partitions,
    queries on the free axis), softmax denominator via a ones-augmented V
    column, normalized context ctx^T [d_local=128, B*S].
  - TWO AllToAlls (one per batch) redistribute ctx^T from head-sharding to
    row-sharding.  The batch-0 AllToAll and its Wo/LayerNorm tail overlap
    with batch-1 attention compute, hiding that collective entirely; only
    the batch-1 AllToAll plus its 2-row-tile tail is exposed at the end.
  - Core c owns rows [256c, 256c+256) of batch 0 AND of batch 1 (512 rows).

The kernel is ScalarEngine-bound: the softmax exp stream (16.8M elements/core)
is ~133 us of ACT time, vs ~88 us of PE time after the fp8 optimizations.
Startup therefore races to the first exp (front loads split across both the
sync- and scalar-engine DMA queues, batch-0 x^T loaded s-chunk-first); the
epilogue splits LayerNorm between DVE (fused scalar_tensor_tensor) and
GpSimd (free after the last collective).

Precision: fp8 (e4m3) matmul inputs with DoubleRow packing (2x PE throughput,
halves the accumulation-matmul count) for the QKV projections, the
probs@V matmul and the Wo projection; bf16 Q^T/K^T for the scores matmul;
fp32 PSUM accumulation, fp32 softmax denominators / residual / LayerNorm.
Scale bookkeeping: x^T is pre-scaled by SX=16 (fp8 subnormal coverage), Q/K
rescale back at the bias-add, V carries SX with a 1/SX denominator column so
normalized context lands at 256*v_avg (fp8 sweet spot), Wo carries 256 and
the residual 65536 -- the common row scale cancels in LayerNorm.

Modeled exec time (MultiCoreSim): 185.8 us vs 264.8 us for the previous
baseline; relative error 1.0e-3 (hardware-verified, gate 2e-2).
"""

import numpy as np
import ml_dtypes

import concourse.bass as bass
import concourse.tile as tile
from concourse import bacc, mybir
from concourse.bass_utils import run_bass_kernel_spmd

BF16 = mybir.dt.bfloat16
FP8 = mybir.dt.float8e4
F32 = mybir.dt.float32
AF = mybir.ActivationFunctionType
DR = mybir.MatmulPerfMode.DoubleRow
P = 128

B, S, H = 2, 2048, 1024
NH, HD = 16, 64
NCORES = 8
EPS = 1e-12
SCALE = 1.0 / 8.0   # 1/sqrt(HD)
SX = 16.0           # fp8 x^T pre-scale (host side)
SVC = 1.0 / SX      # denominator column constant: cn = 256 * v_avg

_CACHE: dict = {}


def _build_program(s=S):
    """Build the (identical-across-cores) Bass program."""
    nkb = s // P               # key blocks of 128 (16)
    qc_per_b = NCORES // B     # q chunks per batch (4)
    qw = (B * s) // NCORES     # q-chunk width (512)
    rpb = s // NCORES          # output rows per core per batch (256)
    ho = H // P                # h chunks of 128 (8)

    nc = bacc.Bacc("TRN2", target_bir_lowering=False, debug=False,
                   num_devices=NCORES)
    xT = nc.dram_tensor("xT", [B, H, s], FP8, kind="ExternalInput")
    wqk = nc.dram_tensor("wqk", [H, 2 * P], FP8, kind="ExternalInput")
    wv = nc.dram_tensor("wv", [H, P], FP8, kind="ExternalInput")
    wo = nc.dram_tensor("wo", [H, H], FP8, kind="ExternalInput")
    bqk = nc.dram_tensor("bqk", [P, 2], F32, kind="ExternalInput")
    bv = nc.dram_tensor("bv", [P, P], F32, kind="ExternalInput")
    maskT = nc.dram_tensor("maskT", [B, P, nkb], F32, kind="ExternalInput")
    xres = nc.dram_tensor("xres", [2 * rpb, H], F32, kind="ExternalInput")
    gamma = nc.dram_tensor("gamma", [P, H], F32, kind="ExternalInput")
    beta = nc.dram_tensor("beta", [P, H], F32, kind="ExternalInput")
    out = nc.dram_tensor("out", [2 * rpb, H], F32, kind="ExternalOutput")

    with tile.TileContext(nc) as tc:
        _kernel_body(
            tc, s, nkb, qw, qc_per_b, rpb, ho,
            xT, wqk, wv, wo, bqk, bv, maskT, xres, gamma, beta, out,
        )
    nc.compile()
    return nc


def _kernel_body(tc, s, nkb, qw, qc_per_b, rpb, ho,
                 xT, wqk, wv, wo, bqk, bv, maskT, xres, gamma, beta, out):
    nc = tc.nc
    VPAD = 80  # padded free width of the ones-augmented V tiles (65 used)
    nkp = nkb // 2  # key-block pairs (8) for DoubleRow AV

    import contextlib
    stack = contextlib.ExitStack()
    with stack:
        consts = stack.enter_context(tc.tile_pool(name="consts", bufs=1))
        dram = stack.enter_context(tc.tile_pool(name="dram", bufs=1, space="DRAM"))

        # ---------------- constant / input loads ----------------
        # Startup is latency-critical (the softmax ScalarEngine stream is the
        # kernel bottleneck, so its first exp should start ASAP).  Split the
        # front loads across BOTH DMA queues: sync (SP) takes wqk + the even
        # x^T chunks, the scalar-engine queue (idle before the first exp)
        # takes bqk/mask + the odd x^T chunks + V-path constants.
        wqk_sb = consts.tile([P, ho, 2, P], FP8)
        wqk_r = wqk.rearrange("(o p) (t d) -> p o t d", p=P, t=2)
        # two DMAs into one tile: the Q half lands first so the very first
        # projection matmul isn't gated on the K half
        nc.sync.dma_start(wqk_sb[:, :, 0, :], wqk_r[:, :, 0, :])
        nc.sync.dma_start(wqk_sb[:, :, 1, :], wqk_r[:, :, 1, :])
        wq_sb = wqk_sb[:, :, 0, :]
        wk_sb = wqk_sb[:, :, 1, :]

        bqk_sb = consts.tile([P, 2], F32)
        nc.scalar.dma_start(bqk_sb, bqk[:, :])
        bq_sb = bqk_sb[:, 0:1]
        bk_sb = bqk_sb[:, 1:2]
        mask_sb = consts.tile([P, B, nkb], F32)
        nc.scalar.dma_start(mask_sb, maskT.rearrange("b p k -> p b k"))

        xT_sb = consts.tile([P, B, ho, s], FP8)
        xT_r = xT.rearrange("b (o p) s -> p b o s", p=P)
        # x^T (fp8, host-scaled by SX): first s-chunk of batch 0 across all o
        for o in range(0, ho, 2):
            nc.sync.dma_start(xT_sb[:, 0, o, 0:512], xT_r[:, 0, o, 0:512])
            nc.scalar.dma_start(
                xT_sb[:, 0, o + 1, 0:512], xT_r[:, 0, o + 1, 0:512])

        wv_sb = consts.tile([P, ho, P], FP8)
        nc.scalar.dma_start(wv_sb, wv.rearrange("(o p) d -> p o d", p=P))
        bv_b = consts.tile([P, P], F32)
        nc.scalar.dma_start(bv_b, bv[:, :])

        # rest of batch-0 x^T, then batch 1 (sync queue; scalar queue must
        # stay clear once the softmax exp stream begins)
        for o in range(ho):
            nc.sync.dma_start(xT_sb[:, 0, o, 512:s], xT_r[:, 0, o, 512:s])
        for o in range(ho):
            nc.sync.dma_start(xT_sb[:, 1, o, :], xT_r[:, 1, o, :])

        wo_sb = consts.tile([P, ho, H], FP8)
        ones_sb = consts.tile([P, P], BF16)
        nc.vector.memset(ones_sb, 1.0)
        eps_sb = consts.tile([P, 1], F32)
        nc.vector.memset(eps_sb, EPS)

        gamma_b = consts.tile([P, H], F32)
        nc.sync.dma_start(gamma_b, gamma[:, :])
        beta_b = consts.tile([P, H], F32)
        nc.sync.dma_start(beta_b, beta[:, :])

        xres_sb = consts.tile([P, 2 * rpb // P, H], F32)

        # attention intermediates
        qT_sb = consts.tile([P, B, s], BF16)   # Q^T [d_local, b, s] true scale
        kT_sb = consts.tile([P, B, s], BF16)   # K^T [d_local, b, s] true scale
        # ones-augmented V (natural layout), per head:
        #   [p(s-inner), b, kb-pair, 2, VPAD] fp8, value scale SX
        v_e = consts.tile([P, B, nkp, 2, VPAD], FP8)
        v_o = consts.tile([P, B, nkp, 2, VPAD], FP8)
        # only the denominator column needs initialization (cols 0:64 are
        # overwritten by the V bias-add; cols 65: are never read):
        # cn = ctx*recip lands at 256*v_avg in fp8 range
        nc.vector.memset(v_e[:, :, :, :, 64:65], SVC)
        nc.vector.memset(v_o[:, :, :, :, 64:65], SVC)

        # A2A bounce buffers (DRAM, local), one pair per HALF-batch: four
        # small collectives let the first three (and their tails) hide under
        # the softmax plateau; only the last 128KB exchange is exposed
        rhb = rpb // 2   # rows per core per half-batch (128)
        a2a_in = [dram.tile([NCORES * P, rhb], FP8, name=f"a2a_in{hb}")
                  for hb in range(2 * B)]
        a2a_out = [dram.tile([NCORES * P, rhb], FP8, name=f"a2a_out{hb}")
                   for hb in range(2 * B)]

        # PSUM: qk pool 1 bank (QKV proj), s pool 2x2 banks, ctx pool 3 banks
        ps_qk = stack.enter_context(tc.tile_pool(name="ps_qk", bufs=1, space="PSUM"))
        ps_s = stack.enter_context(tc.tile_pool(name="ps_s", bufs=2, space="PSUM"))
        ps_ctx = stack.enter_context(tc.tile_pool(name="ps_ctx", bufs=3, space="PSUM"))
        ptile = stack.enter_context(tc.tile_pool(name="ptile", bufs=6))
        misc = stack.enter_context(tc.tile_pool(name="misc", bufs=3))
        fin = stack.enter_context(tc.tile_pool(name="fin", bufs=2))

        def qkv_stage(b):
            # Q/K chunk projections first (attention's scores need them
            # immediately); V blocks after (first consumed only after the
            # first softmax exp)
            for sc in range(s // 512):
                sl = slice(sc * 512, (sc + 1) * 512)
                for w_sb, bias_sb, dst in (
                    (wq_sb, bq_sb, qT_sb),
                    (wk_sb, bk_sb, kT_sb),
                ):
                    ps = ps_qk.tile([P, 512], F32, tag="qk")
                    for op in range(ho // 2):
                        nc.tensor.matmul(
                            ps, lhsT=w_sb[:, 2 * op:2 * op + 2, :].opt(),
                            rhs=xT_sb[:, b, 2 * op:2 * op + 2, sl],
                            start=(op == 0), stop=(op == ho // 2 - 1),
                            perf_mode=DR)
                    # psum is SX * (q|k); rescale to true and add bias
                    nc.vector.scalar_tensor_tensor(
                        dst[:, b, sl], ps, 1.0 / SX,
                        bias_sb[:, 0:1].to_broadcast((P, 512)),
                        mybir.AluOpType.mult, mybir.AluOpType.add)
            for kb in range(nkb):
                ksl = slice(kb * P, (kb + 1) * P)
                ps = ps_qk.tile([P, 512], F32, tag="qk")
                for op in range(ho // 2):
                    nc.tensor.matmul(
                        ps[:, 0:P],
                        lhsT=xT_sb[:, b, 2 * op:2 * op + 2, ksl],
                        rhs=wv_sb[:, 2 * op:2 * op + 2, :],
                        start=(op == 0), stop=(op == ho // 2 - 1),
                        perf_mode=DR)
                # v tiles hold SX * (v + bv); host pre-scales bv by SX
                nc.vector.tensor_tensor(
                    v_e[:, b, kb // 2, kb % 2, 0:64], ps[:, 0:64],
                    bv_b[:, 0:64], mybir.AluOpType.add)
                nc.vector.tensor_tensor(
                    v_o[:, b, kb // 2, kb % 2, 0:64], ps[:, 64:128],
                    bv_b[:, 64:128], mybir.AluOpType.add)

        def attn_stage(b, qcs):
            for qc in qcs:
                qsl = slice(qc * qw, (qc + 1) * qw)
                ctx_e = ps_ctx.tile([P, qw], F32, tag="ctx")
                ctx_o = ps_ctx.tile([P, qw], F32, tag="ctx")
                for kp in range(nkp):
                    pp = ptile.tile([P, 2, 2, qw], FP8, tag="p")
                    for ki in range(2):
                        kb = 2 * kp + ki
                        ksl = slice(kb * P, (kb + 1) * P)
                        sp = ps_s.tile([P, 2, qw], F32, tag="s")
                        nc.tensor.matmul(
                            sp[:, 0, :], lhsT=kT_sb[0:64, b, ksl],
                            rhs=qT_sb[0:64, b, qsl], start=True, stop=True)
                        nc.tensor.matmul(
                            sp[:, 1, :], lhsT=kT_sb[64:128, b, ksl],
                            rhs=qT_sb[64:128, b, qsl], start=True, stop=True)
                        nc.scalar.activation(
                            pp[:, ki], sp, AF.Exp,
                            bias=mask_sb[:, b, kb:kb + 1], scale=SCALE)
                    nc.tensor.matmul(
                        ctx_e[0:65, :], lhsT=v_e[:, b, kp, :, 0:65],
                        rhs=pp[:, :, 0, :], start=(kp == 0), stop=(kp == nkp - 1),
                        perf_mode=DR, skip_group_check=True)
                    nc.tensor.matmul(
                        ctx_o[0:65, :], lhsT=v_o[:, b, kp, :, 0:65],
                        rhs=pp[:, :, 1, :], start=(kp == 0), stop=(kp == nkp - 1),
                        perf_mode=DR, skip_group_check=True)
                # normalize: ctx[d, q] / denom[q]  (denom = row 64)
                last = (b == B - 1 and qc == qc_per_b - 1)
                for h, ctx_ps in enumerate((ctx_e, ctx_o)):
                    rb = misc.tile([1, qw], BF16, tag="rb")
                    with nc.allow_low_precision(reason="bf16 1/denom, matches prior bf16 cast"):
                        nc.vector.reciprocal(rb, ctx_ps[64:65, :])
                    eb = ps_ctx.tile([64, qw], F32, tag="ctx")
                    nc.tensor.matmul(
                        eb, lhsT=ones_sb[0:1, 0:64], rhs=rb,
                        start=True, stop=True)
                    ctx_bf = misc.tile([64, qw], BF16, tag="cb")
                    if last:
                        # final shard is latency-critical (gates the last
                        # collective); ScalarE is idle after the last exp
                        nc.scalar.copy(out=ctx_bf, in_=ctx_ps[0:64, :])
                    else:
                        nc.vector.tensor_copy(out=ctx_bf, in_=ctx_ps[0:64, :])
                    cn = misc.tile([64, 4, rhb], FP8, tag="cn")
                    nc.vector.tensor_tensor(
                        cn, ctx_bf, eb, mybir.AluOpType.mult)
                    # shard (b, qc) holds half-batch hb = 2b + qc//2, cols for
                    # dest cores 4*(qc%2) .. 4*(qc%2)+3; one DMA covers all 4
                    hb = 2 * b + qc // 2
                    d0 = 4 * (qc % 2)
                    nc.sync.dma_start(
                        a2a_in[hb].rearrange(
                            "(d p) q -> p d q", p=P
                        )[h * 64:(h + 1) * 64, d0:d0 + 4, :],
                        cn)

        def tail_stage(hb, ctxf, use_pool):
            # Wo + residual + LayerNorm for this half-batch's 128-row tile
            nc.sync.dma_start(
                ctxf, a2a_out[hb].rearrange("(o p) q -> p o q", p=P))
            if True:
                rt = hb                    # row-tile index in [0, 4)
                qt = 0
                res = fin.tile([P, H], F32, tag="res")
                for nch in range(H // 512):
                    nsl = slice(nch * 512, (nch + 1) * 512)
                    ps = ps_ctx.tile([P, 512], F32, tag="ctx")
                    for op in range(ho // 2):
                        nc.tensor.matmul(
                            ps,
                            lhsT=ctxf[:, 2 * op:2 * op + 2, qt * P:(qt + 1) * P],
                            rhs=wo_sb[:, 2 * op:2 * op + 2, nsl],
                            start=(op == 0), stop=(op == ho // 2 - 1),
                            perf_mode=DR)
                    nc.vector.tensor_tensor(
                        res[:, nsl], ps, xres_sb[:, rt, nsl],
                        mybir.AluOpType.add)
                # LayerNorm over H (free axis)
                stats = fin.tile([P, H // 512, 6], F32, tag="st")
                for g in range(H // 512):
                    nc.vector.bn_stats(
                        stats[:, g, :], res[:, g * 512:(g + 1) * 512])
                mv = fin.tile([P, 2], F32, tag="mv")
                nc.vector.bn_aggr(out=mv, in_=stats)
                rstd = fin.tile([P, 1], F32, tag="rstd")
                nc.scalar.activation(rstd, mv[:, 1:2], AF.Sqrt, bias=eps_sb)
                nc.vector.reciprocal(rstd, rstd)
                u = fin.tile([P, H], F32, tag="u")
                if use_pool:
                    # column halves: Pool's epilogue on half 0 overlaps DVE's
                    # half 1
                    for g in range(2):
                        gs = slice(g * (H // 2), (g + 1) * (H // 2))
                        nc.vector.scalar_tensor_tensor(
                            u[:, gs], res[:, gs], mv[:, 0:1], gamma_b[:, gs],
                            mybir.AluOpType.subtract, mybir.AluOpType.mult)
                else:
                    nc.vector.scalar_tensor_tensor(
                        u, res, mv[:, 0:1], gamma_b,
                        mybir.AluOpType.subtract, mybir.AluOpType.mult)
                outt = fin.tile([P, H], F32, tag="outt")
                if use_pool:
                    # latency-critical tail after the last collective: Pool is
                    # free then, so give it the epilogue in column halves,
                    # each half's store overlapping the next half's compute
                    for g in range(2):
                        gs = slice(g * (H // 2), (g + 1) * (H // 2))
                        nc.gpsimd.tensor_scalar(
                            outt[:, gs], u[:, gs], rstd[:, 0:1], None,
                            mybir.AluOpType.mult)
                        nc.gpsimd.tensor_tensor(
                            outt[:, gs], outt[:, gs], beta_b[:, gs],
                            mybir.AluOpType.add)
                        eng = nc.sync if g == 0 else nc.scalar
                        eng.dma_start(
                            out[rt * P:(rt + 1) * P, gs], outt[:, gs])
                else:
                    # overlapped tail (collective occupies Pool): fused DVE op
                    nc.vector.scalar_tensor_tensor(
                        outt, u, rstd[:, 0:1], beta_b,
                        mybir.AluOpType.mult, mybir.AluOpType.add)
                    nc.sync.dma_start(out[rt * P:(rt + 1) * P, :], outt)

        ctxf = [consts.tile([P, ho, rhb], FP8, name=f"ctxf{hb}", tag=f"ctxf{hb}")
                for hb in range(2 * B)]

        def a2a(hb):
            nc.gpsimd.collective_compute(
                "AllToAll", mybir.AluOpType.bypass,
                replica_groups=[list(range(NCORES))],
                ins=[a2a_in[hb][:].opt()], outs=[a2a_out[hb][:].opt()])

        qkv_stage(0)
        attn_stage(0, (0, 1))
        a2a(0)
        # tail-stage inputs (prefetched well before the first tail runs)
        nc.sync.dma_start(wo_sb, wo.rearrange("(o p) n -> p o n", p=P))
        nc.sync.dma_start(xres_sb, xres.rearrange("(r p) h -> p r h", p=P))
        attn_stage(0, (2, 3))
        a2a(1)
        qkv_stage(1)
        attn_stage(1, (0, 1))
        a2a(2)
        attn_stage(1, (2, 3))
        a2a(3)

        # tails emitted last: their ACT sqrt stays behind every softmax exp,
        # and they gap-fill idle engines / overlap the in-flight collectives
        tail_stage(0, ctxf[0], use_pool=False)
        tail_stage(1, ctxf[1], use_pool=False)
        tail_stage(2, ctxf[2], use_pool=False)
        tail_stage(3, ctxf[3], use_pool=True)


def get_program(s=S):
    key = ("nc", s)
    if key not in _CACHE:
        _CACHE[key] = _build_program(s)
    return _CACHE[key]


def make_in_maps(hidden_states, attention_mask, Wq, bq, Wk, bk, Wv, bv, Wo, bo,
                 ln_gamma, ln_beta):
    """Host-side sharding: build the 8 per-core input maps."""
    f8 = ml_dtypes.float8_e4m3
    hs = np.asarray(hidden_states, dtype=np.float32)
    b_, s_, h_ = hs.shape
    nkb = s_ // P
    rpb = s_ // NCORES

    xT = np.ascontiguousarray(hs.transpose(0, 2, 1) * SX).astype(f8)  # [B,H,S]
    Wq = np.asarray(Wq, np.float32)
    Wk = np.asarray(Wk, np.float32)
    Wv = np.asarray(Wv, np.float32)
    wo_f8 = np.ascontiguousarray(
        np.asarray(Wo, np.float32) * 256.0).astype(f8)
    bq = np.asarray(bq, np.float32)
    bk = np.asarray(bk, np.float32)
    bv = np.asarray(bv, np.float32) * SX
    bo = np.asarray(bo, np.float32)
    gamma_bc = np.ascontiguousarray(
        np.broadcast_to(np.asarray(ln_gamma, np.float32)[None, :], (P, H)))
    beta_bc = np.ascontiguousarray(
        np.broadcast_to(np.asarray(ln_beta, np.float32)[None, :], (P, H)))
    mask = np.asarray(attention_mask, np.float32).reshape(b_, s_)
    maskT = np.ascontiguousarray(
        mask.reshape(b_, nkb, P).transpose(0, 2, 1))  # [B, P, nkb]

    in_maps = []
    hpb = s_ // 2   # rows per half-batch (1024)
    rhb = hpb // NCORES
    for c in range(NCORES):
        d0 = c * P
        xres_c = np.concatenate(
            [hs[hb // 2, (hb % 2) * hpb + c * rhb:(hb % 2) * hpb + (c + 1) * rhb, :]
             for hb in range(4)], axis=0)
        in_maps.append({
            "xT": xT,
            "wqk": np.ascontiguousarray(np.concatenate(
                [Wq[:, d0:d0 + P], Wk[:, d0:d0 + P]], axis=1)).astype(f8),
            "wv": np.ascontiguousarray(Wv[:, d0:d0 + P]).astype(f8),
            "wo": wo_f8,
            "bqk": np.ascontiguousarray(np.stack(
                [bq[d0:d0 + P], bk[d0:d0 + P]], axis=1)),
            "bv": np.ascontiguousarray(
                np.broadcast_to(bv[d0:d0 + P][None, :], (P, P))),
            "maskT": maskT,
            "xres": np.ascontiguousarray((xres_c + bo[None, :]) * 65536.0),
            "gamma": gamma_bc,
            "beta": beta_bc,
        })
    return in_maps


def assemble_output(results, b_=B, s_=S, h_=H):
    hpb = s_ // 2
    rhb = hpb // NCORES
    out = np.empty((b_, s_, h_), np.float32)
    for c in range(NCORES):
        r = np.asarray(results[c]["out"], np.float32)
        for hb in range(4):
            r0 = (hb % 2) * hpb + c * rhb
            out[hb // 2, r0:r0 + rhb, :] = r[hb * rhb:(hb + 1) * rhb]
    return out


def kernel(**inputs):
    nc = get_program(S)
    in_maps = make_in_maps(**inputs)
    res = run_bass_kernel_spmd(nc, in_maps, list(range(NCORES)))
    return assemble_output(res.results)
